# revision 12
# baseline (speedup 1.0000x reference)
"""Bass/Trainium2 kernel for ExtendedTripletLoss (data-parallel over batch).

fp8 DoubleRow design. Math per pair (f1,m1),(f2,m2), shift off in [-4,4]:
  num(off) = t1 + t2 - 2*t3
    t1 = corr(A, m2)(off),  A  = sum_c (m1*f1)^2   [32,512]  (host, f64)
    t2 = corr(m1, B2)(off), B2 = sum_c (m2*f2)^2   [32,512]  (host, f64)
    t3 = corr(U, V/-2)(off), U = m1*f1, V = -2*m2*f2   (device fp8 gram)
  den(off) = C * corr(m1, m2)(off) + 1e-3              (host)
t1/t2/den involve only [32,512]-sized derived tensors; the O(C*H*W)
cross-correlation t3 runs on device as fp8e4 DoubleRow Gram matmuls.

Device, per sample, accumulates PSUM[120, 256] over 5 w-blocks
(4x120 + 32) with 128-wide windows; rhs packs both pairs interleaved
along columns (col,q) and 2 contraction k-tiles per DoubleRow matmul.
Host extracts the 9 lag diagonals col = m + 4 + off.
"""

import os
import sys
from contextlib import ExitStack

import numpy as np

for _p in ("/opt/trn_rl_repo", "/root/.axon_site/_ro/trn_rl_repo"):
    if os.path.isdir(_p) and _p not in sys.path:
        sys.path.insert(0, _p)
        break

import ml_dtypes

import concourse.bass as bass
import concourse.mybir as mybir
import concourse.tile as tile

# This environment's walrus_driver allows only ONE sync-wait per instruction,
# while Tile freely aggregates several. Post-pass: move excess waits onto
# freshly inserted same-engine NOPs directly before the instruction.
_MAXW = 1


def _split_waits_pass(nc):
    n = 0
    for fn in nc.m.functions:
        for blk in fn.blocks:
            out = []
            changed = False
            for inst in blk.instructions:
                si = inst.sync_info
                waits = list(si.on_wait) if si is not None else []
                if len(waits) > _MAXW:
                    for i in range(0, len(waits) - _MAXW, _MAXW):
                        nop = mybir.InstNoOp(name=f"{inst.name}-wsplit{i}")
                        nop.engine = inst.engine
                        nop.sync_info = mybir.SyncInfo(
                            on_update=[], on_wait=waits[i : i + _MAXW]
                        )
                        out.append(nop)
                        n += 1
                    si.on_wait = waits[len(waits) - _MAXW :]
                    changed = True
                out.append(inst)
            if changed:
                blk.instructions = out
    return n


FP8 = mybir.dt.float8e4
BF16 = mybir.dt.bfloat16
F32 = mybir.dt.float32
NPFP8 = ml_dtypes.float8_e4m3
NPBF16 = ml_dtypes.bfloat16

B, C, H, W = 64, 16, 32, 512
NCORES = 8
S = B // NCORES          # samples per core
R = C * H                # 512 rows in (c,h) contraction dim
NB = R // 128            # 4 partition chunks
BLK = 120                # w-block width; 5 blocks: 4x120 + 32
WIN = 128                # window width for full blocks (BLK + 2*SHIFT)
VW = W + 8               # padded V width
MARGIN = 0.15
SHIFT = 4

_nc_cache = None


def build_nc(for_hw=True):
    DR = mybir.MatmulPerfMode.DoubleRow
    nc = bass.Bass()
    # Per-sample blob, one half per DoubleRow k-tile pair t:
    # x_b[s, part, t, kc, 0:512]    = U[2t+kc]     (masked anchor)
    # x_b[s, part, t, kc, 512:1552] = Vpad[2t+kc]  (-2*masked p|n, (w,q) flat)
    x_b = nc.declare_dram_parameter("x_b", [S, 128, 2, 2, W + 2 * VW], FP8, isOutput=False)
    # raw[s, m, (n,q)]: accumulated -2*t3 gram blocks; diagonals on host
    raw = nc.declare_dram_parameter("raw", [S, BLK, 256], BF16, isOutput=True)

    with tile.TileContext(nc) as tc, ExitStack() as ctx:
        # all 8 samples resident: DMA stream fully decoupled from PE
        io = ctx.enter_context(tc.tile_pool(name="io", bufs=S))
        outsb = ctx.enter_context(tc.tile_pool(name="outsb", bufs=S))
        gram = ctx.enter_context(tc.tile_pool(name="gram", bufs=4, space="PSUM"))

        if for_hw:
            # PE p-state prewarm with NO data dependency (reads whatever is
            # in SBUF): starts right after program load, never gates real
            # work, and ramps the clock before sample 0's data lands.
            warmps = ctx.enter_context(tc.tile_pool(name="warm", bufs=1, space="PSUM"))
            junk = ctx.enter_context(tc.tile_pool(name="junk", bufs=1))
            jt = junk.tile([128, 512], FP8)
            nc.vector.memset(jt, 1.0)
            wp = warmps.tile([32, 512], F32)
            for _ in range(4):
                nc.tensor.matmul(wp, jt[:, 0:32], jt, start=True, stop=True)

        # ---- prefetch burst: all input DMAs back-to-back on both HWDGE
        # queues, before any compute instruction occupies them; sample 0's
        # halves are split so its first matmuls start sooner ----
        blobs = []
        for s in range(S):
            blob = io.tile([128, 2, 2, W + 2 * VW], FP8, tag="blob")
            if s == 0:
                # t0 in two column-prefix pieces: piece A covers the first
                # two j-blocks' lhsT+rhs ranges, so matmuls start ~0.7us
                # before the full half lands
                nc.sync.dma_start(out=blob[:, 0, :, 0:1024], in_=x_b[s, :, 0, :, 0:1024])
                nc.sync.dma_start(out=blob[:, 0, :, 1024:], in_=x_b[s, :, 0, :, 1024:])
                nc.scalar.dma_start(out=blob[:, 1], in_=x_b[s, :, 1])
            else:
                nc.sync.dma_start(out=blob[:, 0], in_=x_b[s, :, 0])
                nc.scalar.dma_start(out=blob[:, 1], in_=x_b[s, :, 1])
            blobs.append(blob)

        for s in range(S):
            blob = blobs[s]
            # ---- 10 DoubleRow matmuls accumulating into one PSUM tile;
            # t-major so the t=0 half starts as soon as its blob lands ----
            num_ps = gram.tile([BLK, 256], F32, tag="num")
            for t in range(2):
                for j in range(5):
                    wj = BLK if j < 4 else 32
                    fw = 2 * (wj + 8)
                    lc = slice(BLK * j, BLK * j + wj)
                    wn = slice(W + 240 * j, W + 240 * j + fw)
                    nc.tensor.matmul(
                        num_ps[0:wj, 0:fw],
                        blob[:, t, :, lc],
                        blob[:, t, :, wn],
                        start=(t == 0 and j == 0),
                        stop=(t == 1 and j == 4),
                        perf_mode=DR,
                        skip_group_check=True,
                    )

            # ---- PSUM -> SBUF (DVE, idle otherwise) -> HBM on the HWDGE
            # queues (no gpsimd: avoids swdge boot + teardown drain) ----
            psb = outsb.tile([BLK, 256], BF16, tag="psb")
            nc.vector.tensor_copy(out=psb, in_=num_ps)
            eng = nc.sync if s % 2 == 0 else nc.scalar
            eng.dma_start(out=raw[s], in_=psb)
    if for_hw:
        _split_waits_pass(nc)
    return nc


def _host_prep(a, p, n, ma, mp, mn):
    a = np.asarray(a, dtype=np.float32)
    p = np.asarray(p, dtype=np.float32)
    n = np.asarray(n, dtype=np.float32)
    mav = np.asarray(ma).reshape(B, H, W)
    mpv = np.asarray(mp).reshape(B, H, W)
    mnv = np.asarray(mn).reshape(B, H, W)

    U = (a * mav.astype(np.float32)[:, None]).reshape(B, NB, 128, W)
    U = np.ascontiguousarray(U.transpose(0, 2, 1, 3)).astype(NPFP8)  # [B,128,NB,W]

    Vp = (p * mpv.astype(np.float32)[:, None]).reshape(B, R, W)
    Vn = (n * mnv.astype(np.float32)[:, None]).reshape(B, R, W)
    V = np.stack([Vp, Vn], axis=-1) * -2.0                      # [B,R,W,2]
    V = V.reshape(B, NB, 128, W, 2).transpose(0, 2, 1, 3, 4)    # [B,128,NB,W,2]
    V8 = V.astype(NPFP8)
    Vpad = np.concatenate([V8[:, :, :, W - 4 :], V8, V8[:, :, :, :4]], axis=3)
    Vflat = Vpad.reshape(B, 128, NB, 2 * VW)

    blob = np.empty((B, 128, 2, 2, W + 2 * VW), NPFP8)
    blob[..., 0:W] = U.reshape(B, 128, 2, 2, W)
    blob[..., W:] = Vflat.reshape(B, 128, 2, 2, 2 * VW)

    in_maps = []
    for c in range(NCORES):
        sl = slice(c * S, (c + 1) * S)
        in_maps.append({"x_b": blob[sl]})
    return in_maps, U, V8


def _host_t12(U, V8, ma, mp, mn):
    """t1+t2 per (b, pair, off), f64, from the quantized U/V for consistency
    with the device gram (num = t1 + t2 - 2*t3 with matching f^2 terms)."""
    nb = U.shape[0]
    mav = np.asarray(ma).reshape(nb, H, W).astype(np.float64)
    m2 = np.stack(
        [np.asarray(mp).reshape(nb, H, W), np.asarray(mn).reshape(nb, H, W)], axis=-1
    ).astype(np.float64)                                        # [nb,H,W,2]

    Uq = U.astype(np.float32).transpose(0, 2, 1, 3).reshape(nb, C, H, W)
    A = (Uq.astype(np.float64) ** 2).sum(axis=1)                # [nb,H,W]
    Vq = V8.astype(np.float32).transpose(0, 2, 1, 3, 4).reshape(nb, C, H, W, 2)
    B2 = (Vq.astype(np.float64) ** 2).sum(axis=1) * 0.25        # [nb,H,W,2]

    t12 = np.empty((nb, 2, 2 * SHIFT + 1), np.float64)
    for i, off in enumerate(range(-SHIFT, SHIFT + 1)):
        m2r = np.roll(m2, off, axis=2)
        b2r = np.roll(B2, off, axis=2)
        t12[:, :, i] = np.einsum("bhw,bhwq->bq", A, m2r) + np.einsum(
            "bhw,bhwq->bq", mav, b2r
        )
    return t12


def _host_den(ma, mp, mn):
    nb = ma.shape[0]
    m1 = np.asarray(ma).reshape(nb, H, W).astype(bool)
    cnts = np.empty((nb, 2, 2 * SHIFT + 1), np.float64)
    for pair, m2 in enumerate((mp, mn)):
        m2 = np.asarray(m2).reshape(nb, H, W).astype(bool)
        for i, off in enumerate(range(-SHIFT, SHIFT + 1)):
            cnts[:, pair, i] = (m1 & np.roll(m2, off, axis=-1)).sum(axis=(1, 2))
    return cnts


def _host_finish(raw_all, cnts, t12):
    # raw_all: [B, BLK, 256] bf16 = -2*t3 blocks; num = t12 + diag sums
    nb = raw_all.shape[0]
    rawv = raw_all.astype(np.float64).reshape(nb, BLK, WIN, 2)
    m_idx = np.arange(BLK)
    dists = []
    for i, off in enumerate(range(-SHIFT, SHIFT + 1)):
        g3 = rawv[:, m_idx, m_idx + 4 + off, :].sum(axis=1)    # [nb, 2]
        num = t12[:, :, i] + g3
        dists.append(num / (C * cnts[:, :, i] + 0.001))
    d = np.min(np.stack(dists, axis=0), axis=0)                # [nb, 2]
    loss = np.maximum(d[:, 0] - d[:, 1] + MARGIN, 0.0)
    return np.array(loss.mean(), dtype=np.float32)


def kernel(a, p, n, ma, mp, mn):
    global _nc_cache
    from concourse import bass_utils

    if _nc_cache is None:
        _nc_cache = build_nc()
    nc = _nc_cache
    in_maps, U, V8 = _host_prep(a, p, n, ma, mp, mn)
    res = bass_utils.run_bass_kernel_spmd(nc, in_maps, core_ids=list(range(NCORES)))
    raw_all = np.concatenate([res.results[i]["raw"] for i in range(NCORES)], axis=0)
    return _host_finish(raw_all, _host_den(ma, mp, mn), _host_t12(U, V8, ma, mp, mn))


# revision 13
# speedup vs baseline: 1.1926x; 1.1926x over previous
"""Bass/Trainium2 kernel for ExtendedTripletLoss (data-parallel over batch).

fp8 DoubleRow design. Math per pair (f1,m1),(f2,m2), shift off in [-4,4]:
  num(off) = t1 + t2 - 2*t3
    t1 = corr(A, m2)(off),  A  = sum_c (m1*f1)^2   [32,512]  (host, f64)
    t2 = corr(m1, B2)(off), B2 = sum_c (m2*f2)^2   [32,512]  (host, f64)
    t3 = corr(U, V/-2)(off), U = m1*f1, V = -2*m2*f2   (device fp8 gram)
  den(off) = C * corr(m1, m2)(off) + 1e-3              (host)
t1/t2/den involve only [32,512]-sized derived tensors; the O(C*H*W)
cross-correlation t3 runs on device as fp8e4 DoubleRow Gram matmuls.

Device, per sample, accumulates PSUM[120, 256] over 5 w-blocks
(4x120 + 32) with 128-wide windows; rhs packs both pairs interleaved
along columns (col,q) and 2 contraction k-tiles per DoubleRow matmul.
Host extracts the 9 lag diagonals col = m + 4 + off.
"""

import os
import sys
from contextlib import ExitStack

import numpy as np

for _p in ("/opt/trn_rl_repo", "/root/.axon_site/_ro/trn_rl_repo"):
    if os.path.isdir(_p) and _p not in sys.path:
        sys.path.insert(0, _p)
        break

import ml_dtypes

import concourse.bass as bass
import concourse.mybir as mybir
import concourse.tile as tile

# This environment's walrus_driver allows only ONE sync-wait per instruction,
# while Tile freely aggregates several. Post-pass: move excess waits onto
# freshly inserted same-engine NOPs directly before the instruction.
_MAXW = 1


def _split_waits_pass(nc):
    n = 0
    for fn in nc.m.functions:
        for blk in fn.blocks:
            out = []
            changed = False
            for inst in blk.instructions:
                si = inst.sync_info
                waits = list(si.on_wait) if si is not None else []
                if len(waits) > _MAXW:
                    for i in range(0, len(waits) - _MAXW, _MAXW):
                        nop = mybir.InstNoOp(name=f"{inst.name}-wsplit{i}")
                        nop.engine = inst.engine
                        nop.sync_info = mybir.SyncInfo(
                            on_update=[], on_wait=waits[i : i + _MAXW]
                        )
                        out.append(nop)
                        n += 1
                    si.on_wait = waits[len(waits) - _MAXW :]
                    changed = True
                out.append(inst)
            if changed:
                blk.instructions = out
    return n


FP8 = mybir.dt.float8e4
BF16 = mybir.dt.bfloat16
F32 = mybir.dt.float32
NPFP8 = ml_dtypes.float8_e4m3
NPBF16 = ml_dtypes.bfloat16

B, C, H, W = 64, 16, 32, 512
NCORES = 8
S = B // NCORES          # samples per core
R = C * H                # 512 rows in (c,h) contraction dim
NB = R // 128            # 4 partition chunks
BLK = 120                # w-block width; 5 blocks: 4x120 + 32
WIN = 128                # window width for full blocks (BLK + 2*SHIFT)
VW = W + 8               # padded V width
MARGIN = 0.15
SHIFT = 4

_nc_cache = None


def build_nc(for_hw=True):
    DR = mybir.MatmulPerfMode.DoubleRow
    nc = bass.Bass()
    # Per-sample blob, one half per DoubleRow k-tile pair t:
    # x_b[s, part, t, kc, 0:512]    = U[2t+kc]     (masked anchor)
    # x_b[s, part, t, kc, 512:1552] = Vpad[2t+kc]  (-2*masked p|n, (w,q) flat)
    x_b = nc.declare_dram_parameter("x_b", [S, 128, 2, 2, W + 2 * VW], FP8, isOutput=False)
    # raw[s, m, (n,q)]: accumulated -2*t3 gram blocks; diagonals on host
    raw = nc.declare_dram_parameter("raw", [S, BLK, 256], BF16, isOutput=True)

    with tile.TileContext(nc) as tc, ExitStack() as ctx:
        # all 8 samples resident: DMA stream fully decoupled from PE
        io = ctx.enter_context(tc.tile_pool(name="io", bufs=S))
        outsb = ctx.enter_context(tc.tile_pool(name="outsb", bufs=S))
        gram = ctx.enter_context(tc.tile_pool(name="gram", bufs=4, space="PSUM"))

        # ---- prefetch burst: all input DMAs back-to-back on both HWDGE
        # queues, before any compute instruction occupies them ----
        blobs = []
        for s in range(S):
            blob = io.tile([128, 2, 2, W + 2 * VW], FP8, tag="blob")
            nc.sync.dma_start(out=blob[:, 0], in_=x_b[s, :, 0])
            nc.scalar.dma_start(out=blob[:, 1], in_=x_b[s, :, 1])
            blobs.append(blob)

        for s in range(S):
            blob = blobs[s]
            # ---- 10 DoubleRow matmuls accumulating into one PSUM tile;
            # t-major so the t=0 half starts as soon as its blob lands ----
            num_ps = gram.tile([BLK, 256], F32, tag="num")
            for t in range(2):
                for j in range(5):
                    wj = BLK if j < 4 else 32
                    fw = 2 * (wj + 8)
                    lc = slice(BLK * j, BLK * j + wj)
                    wn = slice(W + 240 * j, W + 240 * j + fw)
                    nc.tensor.matmul(
                        num_ps[0:wj, 0:fw],
                        blob[:, t, :, lc],
                        blob[:, t, :, wn],
                        start=(t == 0 and j == 0),
                        stop=(t == 1 and j == 4),
                        perf_mode=DR,
                        skip_group_check=True,
                    )

            # ---- PSUM -> SBUF (DVE, idle otherwise) -> HBM on the HWDGE
            # queues (no gpsimd: avoids swdge boot + teardown drain) ----
            psb = outsb.tile([BLK, 256], BF16, tag="psb")
            nc.vector.tensor_copy(out=psb, in_=num_ps)
            eng = nc.sync if s % 2 == 0 else nc.scalar
            eng.dma_start(out=raw[s], in_=psb)
    if for_hw:
        _split_waits_pass(nc)
    return nc


def _host_prep(a, p, n, ma, mp, mn):
    a = np.asarray(a, dtype=np.float32)
    p = np.asarray(p, dtype=np.float32)
    n = np.asarray(n, dtype=np.float32)
    mav = np.asarray(ma).reshape(B, H, W)
    mpv = np.asarray(mp).reshape(B, H, W)
    mnv = np.asarray(mn).reshape(B, H, W)

    U = (a * mav.astype(np.float32)[:, None]).reshape(B, NB, 128, W)
    U = np.ascontiguousarray(U.transpose(0, 2, 1, 3)).astype(NPFP8)  # [B,128,NB,W]

    Vp = (p * mpv.astype(np.float32)[:, None]).reshape(B, R, W)
    Vn = (n * mnv.astype(np.float32)[:, None]).reshape(B, R, W)
    V = np.stack([Vp, Vn], axis=-1) * -2.0                      # [B,R,W,2]
    V = V.reshape(B, NB, 128, W, 2).transpose(0, 2, 1, 3, 4)    # [B,128,NB,W,2]
    V8 = V.astype(NPFP8)
    Vpad = np.concatenate([V8[:, :, :, W - 4 :], V8, V8[:, :, :, :4]], axis=3)
    Vflat = Vpad.reshape(B, 128, NB, 2 * VW)

    blob = np.empty((B, 128, 2, 2, W + 2 * VW), NPFP8)
    blob[..., 0:W] = U.reshape(B, 128, 2, 2, W)
    blob[..., W:] = Vflat.reshape(B, 128, 2, 2, 2 * VW)

    in_maps = []
    for c in range(NCORES):
        sl = slice(c * S, (c + 1) * S)
        in_maps.append({"x_b": blob[sl]})
    return in_maps, U, V8


def _host_t12(U, V8, ma, mp, mn):
    """t1+t2 per (b, pair, off), f64, from the quantized U/V for consistency
    with the device gram (num = t1 + t2 - 2*t3 with matching f^2 terms)."""
    nb = U.shape[0]
    mav = np.asarray(ma).reshape(nb, H, W).astype(np.float64)
    m2 = np.stack(
        [np.asarray(mp).reshape(nb, H, W), np.asarray(mn).reshape(nb, H, W)], axis=-1
    ).astype(np.float64)                                        # [nb,H,W,2]

    Uq = U.astype(np.float32).transpose(0, 2, 1, 3).reshape(nb, C, H, W)
    A = (Uq.astype(np.float64) ** 2).sum(axis=1)                # [nb,H,W]
    Vq = V8.astype(np.float32).transpose(0, 2, 1, 3, 4).reshape(nb, C, H, W, 2)
    B2 = (Vq.astype(np.float64) ** 2).sum(axis=1) * 0.25        # [nb,H,W,2]

    t12 = np.empty((nb, 2, 2 * SHIFT + 1), np.float64)
    for i, off in enumerate(range(-SHIFT, SHIFT + 1)):
        m2r = np.roll(m2, off, axis=2)
        b2r = np.roll(B2, off, axis=2)
        t12[:, :, i] = np.einsum("bhw,bhwq->bq", A, m2r) + np.einsum(
            "bhw,bhwq->bq", mav, b2r
        )
    return t12


def _host_den(ma, mp, mn):
    nb = ma.shape[0]
    m1 = np.asarray(ma).reshape(nb, H, W).astype(bool)
    cnts = np.empty((nb, 2, 2 * SHIFT + 1), np.float64)
    for pair, m2 in enumerate((mp, mn)):
        m2 = np.asarray(m2).reshape(nb, H, W).astype(bool)
        for i, off in enumerate(range(-SHIFT, SHIFT + 1)):
            cnts[:, pair, i] = (m1 & np.roll(m2, off, axis=-1)).sum(axis=(1, 2))
    return cnts


def _host_finish(raw_all, cnts, t12):
    # raw_all: [B, BLK, 256] bf16 = -2*t3 blocks; num = t12 + diag sums
    nb = raw_all.shape[0]
    rawv = raw_all.astype(np.float64).reshape(nb, BLK, WIN, 2)
    m_idx = np.arange(BLK)
    dists = []
    for i, off in enumerate(range(-SHIFT, SHIFT + 1)):
        g3 = rawv[:, m_idx, m_idx + 4 + off, :].sum(axis=1)    # [nb, 2]
        num = t12[:, :, i] + g3
        dists.append(num / (C * cnts[:, :, i] + 0.001))
    d = np.min(np.stack(dists, axis=0), axis=0)                # [nb, 2]
    loss = np.maximum(d[:, 0] - d[:, 1] + MARGIN, 0.0)
    return np.array(loss.mean(), dtype=np.float32)


def kernel(a, p, n, ma, mp, mn):
    global _nc_cache
    from concourse import bass_utils

    if _nc_cache is None:
        _nc_cache = build_nc()
    nc = _nc_cache
    in_maps, U, V8 = _host_prep(a, p, n, ma, mp, mn)
    res = bass_utils.run_bass_kernel_spmd(nc, in_maps, core_ids=list(range(NCORES)))
    raw_all = np.concatenate([res.results[i]["raw"] for i in range(NCORES)], axis=0)
    return _host_finish(raw_all, _host_den(ma, mp, mn), _host_t12(U, V8, ma, mp, mn))
